# revision 8
# baseline (speedup 1.0000x reference)
"""AttentionVAE Trainium2 kernel: 8-core data-parallel Bass/Tile implementation.

Layout strategy: feature-major activations [F, batch_free] on-chip.
Host pre-transposes x -> [196, B] and post-transposes outputs back, so all
device DMA is contiguous.  All four decoder invocations (recon + 3 per-layer
losses) run fused as one 4-stream block-diagonal decoder.  Sigmoid is
rewritten as 0.5*tanh(x/2)+0.5 (fused into consumers) so every activation
function lives in one ACT table set (exp_and_others) -> no table reloads.
Matmuls run as float32r (full PE rate at N=512).
"""
import sys

sys.path.insert(0, "/opt/trn_rl_repo")

from contextlib import ExitStack

import numpy as np
import ml_dtypes

BF = ml_dtypes.bfloat16
MM_KEYS = {"Wfa1a", "Wfa1b", "Wfa2h", "Wfa2l", "onesA", "onesB", "We1a",
           "We1b", "We2", "We3", "Wmv", "Wd1x", "Wd2x", "Wd3x", "Wd4h",
           "Wd4l", "Wd4_96", "Wd4_64", "Wd4_32", "O96", "O64", "O32",
           "Wla", "onesE"}

D = 196
B = 262144
NCORES = 8
BC = B // NCORES          # 32768 samples per core
NB = 1024                 # batch tile (free dim) per outer iteration
NT = BC // NB             # 16 tiles
NC_CH = NB // 512         # 4 matmul chunks of 512
ALPHA = 0.1

_BUILT = None


def _prep_host(inputs):
    """Weight/constant blobs in the layouts the kernel wants (all float32)."""
    f = lambda k: np.asarray(inputs[k], np.float32)
    w = {}
    fa_w1, fa_b1 = f("fa_w1"), f("fa_b1")
    fa_w2, fa_b2 = f("fa_w2"), f("fa_b2")
    w["Wfa1a"] = fa_w1[0:128].copy()
    w["Wfa1b"] = fa_w1[128:196].copy()
    w["bfa1"] = fa_b1.reshape(98, 1).copy()
    w["Wfa2h"] = fa_w2[:, 0:128].copy()
    w["Wfa2l"] = fa_w2[:, 128:196].copy()
    w["bfa2h"] = (0.5 * fa_b2[0:128]).reshape(128, 1).copy()
    w["bfa2l"] = (0.5 * fa_b2[128:196]).reshape(68, 1).copy()
    # mean-of-a broadcast matmul: psum = 2*m = 1 + (1/196)*sum(th2).
    w["onesA"] = np.full((128, 128), 1.0 / D, np.float32)
    onesB = np.full((69, 128), 1.0 / D, np.float32)
    onesB[68, :] = 1.0  # const-1 row of th2_lo contributes the "+1"
    w["onesB"] = onesB
    e_w1, e_b1 = f("e_w1"), f("e_b1")
    w["We1a"] = e_w1[0:128].copy()
    w["We1b"] = e_w1[128:196].copy()
    w["be1"] = e_b1.reshape(96, 1).copy()
    w["We2"] = f("e_w2")
    w["be2"] = f("e_b2").reshape(64, 1).copy()
    w["We3"] = f("e_w3")
    w["be3"] = f("e_b3").reshape(32, 1).copy()
    e_wm, e_bm = f("e_wm"), f("e_bm")
    e_wv, e_bv = f("e_wv"), f("e_bv")
    # cols 0:16 -> [mean|logvar] outputs; 16:48 -> mean x4; 48:80 -> logvar x4
    Wmv = np.zeros((32, 96), np.float32)
    Wmv[:, 0:8] = e_wm
    Wmv[:, 8:16] = e_wv
    Wmv[:, 32:40] = e_wm
    Wmv[:, 64:96] = np.tile(e_wv, (1, 4))
    w["Wmv"] = Wmv
    w["bmv"] = np.concatenate([e_bm, e_bv]).reshape(16, 1).copy()
    w["bv4h"] = (0.5 * np.tile(e_bv, 4)).reshape(32, 1).copy()
    d_w1, d_b1 = f("d_w1"), f("d_b1")
    d_w2, d_b2 = f("d_w2"), f("d_b2")
    d_w3, d_b3 = f("d_w3"), f("d_b3")
    d_w4, d_b4 = f("d_w4"), f("d_b4")
    # 4-stream fused decoder layer 1: rhs rows 0:8 = mean, 8:40 = eps*std x4
    Wd1x = np.zeros((40, 128), np.float32)
    for k in range(4):
        Wd1x[8 * k : 8 * (k + 1), 32 * k : 32 * (k + 1)] = d_w1
    Wd1x[32:40, :] = np.tile(d_w1, (1, 4))
    w["Wd1x"] = Wd1x
    b1eff = d_b1 + e_bm @ d_w1
    w["bd1x"] = np.tile(b1eff, 4).reshape(128, 1).copy()
    Wd2x = np.zeros((64, 128), np.float32)
    Wd2x[0:32, 0:64] = d_w2
    Wd2x[32:64, 64:128] = d_w2
    w["Wd2x"] = np.vstack([Wd2x, Wd2x])          # [128,128] dup for base 0/64
    w["bd2x"] = np.tile(d_b2, 2).reshape(128, 1).copy()
    w["Wd3x"] = np.vstack([d_w3, d_w3])           # [128,96] dup for base 0/64
    w["bd3"] = d_b3.reshape(96, 1).copy()
    w["Wd4h"] = d_w4[:, 0:128].copy()
    w["Wd4l"] = d_w4[:, 128:196].copy()
    w["brech"] = (0.5 * d_b4[0:128]).reshape(128, 1).copy()
    w["brecl"] = (0.5 * d_b4[128:196]).reshape(68, 1).copy()
    w["Wd4_96"] = d_w4[:, 0:96].copy()
    w["bl1h"] = (0.5 * d_b4[0:96]).reshape(96, 1).copy()
    w["Wd4_64"] = d_w4[:, 0:64].copy()
    w["Wd4_32"] = d_w4[:, 0:32].copy()
    w["bl23h"] = (0.5 * np.concatenate([d_b4[0:64], d_b4[0:32]])).reshape(96, 1).copy()
    O96 = np.zeros((96, 3), np.float32); O96[:, 0] = 1.0 / 96
    O64 = np.zeros((64, 3), np.float32); O64[:, 1] = 1.0 / 64
    O32 = np.zeros((96, 3), np.float32); O32[64:96, 2] = 1.0 / 32
    w["O96"] = O96; w["O64"] = O64; w["O32"] = O32
    w["bneg05"] = np.full((96, 1), -0.5, np.float32)
    w["Wla"] = f("la_w")
    w["blah"] = (0.5 * f("la_b")).reshape(3, 1).copy()
    onesE = np.zeros((35, 33), np.float32)
    onesE[0:3, 0] = 1.0
    onesE[32:35, 32] = 1.0
    w["onesE"] = onesE
    for k in MM_KEYS:
        w[k] = w[k].astype(BF)
    return w


def _build():
    import concourse.bacc as bacc
    import concourse.tile as tile
    from concourse import mybir

    nc = bacc.Bacc("TRN2", target_bir_lowering=False, debug=False,
                   num_devices=NCORES)
    dt = mybir.dt
    F32 = dt.float32
    B16 = dt.bfloat16
    AF = mybir.ActivationFunctionType
    ALU = mybir.AluOpType

    xT = nc.dram_tensor("xT", [D, BC], B16, kind="ExternalInput").ap()
    epsT = nc.dram_tensor("epsT", [32, BC], B16, kind="ExternalInput").ap()

    WSPECS = {
        "Wfa1a": (128, 98), "Wfa1b": (68, 98), "bfa1": (98, 1),
        "Wfa2h": (98, 128), "Wfa2l": (98, 68), "bfa2h": (128, 1), "bfa2l": (68, 1),
        "onesA": (128, 128), "onesB": (69, 128),
        "We1a": (128, 96), "We1b": (68, 96), "be1": (96, 1),
        "We2": (96, 64), "be2": (64, 1), "We3": (64, 32), "be3": (32, 1),
        "Wmv": (32, 96), "bmv": (16, 1), "bv4h": (32, 1),  # Wmv lives at rows 64:96 of a [96,80] tile
        "Wd1x": (40, 128), "bd1x": (128, 1), "Wd2x": (128, 128), "bd2x": (128, 1),
        "Wd3x": (128, 96), "bd3": (96, 1),
        "Wd4h": (96, 128), "Wd4l": (96, 68), "brech": (128, 1), "brecl": (68, 1),
        "Wd4_96": (96, 96), "bl1h": (96, 1),
        "Wd4_64": (96, 64), "Wd4_32": (96, 32), "bl23h": (96, 1),
        "O96": (96, 3), "O64": (64, 3), "O32": (96, 3), "bneg05": (96, 1),
        "Wla": (3, 3), "blah": (3, 1), "onesE": (35, 33),
    }
    wdram = {k: nc.dram_tensor(k, list(s), B16 if k in MM_KEYS else F32,
                               kind="ExternalInput").ap()
             for k, s in WSPECS.items()}

    attnT = nc.dram_tensor("attnT", [D, BC], F32, kind="ExternalOutput").ap()
    reconT = nc.dram_tensor("reconT", [D, BC], F32, kind="ExternalOutput").ap()
    meanT = nc.dram_tensor("meanT", [8, BC], F32, kind="ExternalOutput").ap()
    logvarT = nc.dram_tensor("logvarT", [8, BC], F32, kind="ExternalOutput").ap()
    wlout = nc.dram_tensor("wlout", [1, 1], F32, kind="ExternalOutput").ap()

    def r(ap):  # matmul operands already bf16
        return ap

    with tile.TileContext(nc) as tc, ExitStack() as ctx:
        P = ctx.enter_context  # pool helper
        wpool = P(tc.tile_pool(name="weights", bufs=1))
        W = {}
        for k, s in WSPECS.items():
            wdt = B16 if k in MM_KEYS else F32
            if k == "Wmv":
                W[k] = wpool.tile([96, 96], wdt, name=k, tag=k)
                nc.sync.dma_start(W[k][64:96, :], wdram[k][:])
            else:
                W[k] = wpool.tile(list(s), wdt, name=k, tag=k)
                nc.sync.dma_start(W[k][:], wdram[k][:])

        # persistent tiles
        const_pool = P(tc.tile_pool(name="const", bufs=1))
        th2_lo = const_pool.tile([96, NB], B16, tag="th2lo")
        nc.vector.memset(th2_lo[64:96, :], 1.0)  # row 68 = const-1 for onesB
        wl_acc = const_pool.tile([1, NT * NC_CH], F32, tag="wlacc")
        junk = const_pool.tile([1, 512], F32, tag="junk")
        est = const_pool.tile([35, NB], B16, tag="est")
        nc.vector.memset(est[0:32, :], 0.0)  # filler rows stay zero

        # sbuf pools
        xin = P(tc.tile_pool(name="xin", bufs=2))
        epool = P(tc.tile_pool(name="eps", bufs=2))
        big1 = P(tc.tile_pool(name="big1", bufs=2))   # [128, NB] rotating
        lo1 = P(tc.tile_pool(name="lo1", bufs=2))     # [68, NB] rotating
        thp = P(tc.tile_pool(name="thp", bufs=1))     # [98/96, NB]
        misc = P(tc.tile_pool(name="misc", bufs=1))
        aout = P(tc.tile_pool(name="aout", bufs=2))
        rout = P(tc.tile_pool(name="rout", bufs=2))
        decp = P(tc.tile_pool(name="decp", bufs=1))
        sqp = P(tc.tile_pool(name="sqp", bufs=1))
        lap = P(tc.tile_pool(name="lap", bufs=1))

        psA = P(tc.tile_pool(name="psA", bufs=3, space="PSUM"))
        psS = P(tc.tile_pool(name="psS", bufs=2, space="PSUM"))

        CH = [(c, slice(c * 512, (c + 1) * 512)) for c in range(NC_CH)]
        # [(psum_tile_idx, slice within psum tile, slice within NB tile)]
        GR = [(g, [(slice(c * 512, (c + 1) * 512),
                    slice((2 * g + c) * 512, (2 * g + c + 1) * 512))
                   for c in range(2)]) for g in range(NC_CH // 2)]

        def mm_stage(name, wkeys_rhs, M, act_fn, out, bias, scale=1.0,
                     alpha=0.0, out_rows=None):
            """K-outer matmul group + ACT evacuation of [M, 1024] psum tiles.
            wkeys_rhs: list of (weight_tile, rhs_tile, rhs_row_slice) accumulated.
            """
            for g, pairs in GR:
                ps = psA.tile([M, 1024], F32, name=f"ps_{name}{g}", tag="psA")
                nk = len(wkeys_rhs)
                for ki, (wap, rhs, rsl) in enumerate(wkeys_rhs):
                    for psl, nsl in pairs:
                        nc.tensor.matmul(ps[:, psl], r(wap), r(rhs[rsl, nsl]),
                                         start=(ki == 0), stop=(ki == nk - 1))
                orows = out_rows if out_rows is not None else slice(0, M)
                nc.scalar.activation(out[orows, (2 * g) * 512:(2 * g + 2) * 512],
                                     ps[:], act_fn, bias=bias, scale=scale,
                                     alpha=alpha)
                yield ps, g

        def run(gen):
            for _ in gen:
                pass

        for t in range(NT):
            cs = slice(t * NB, (t + 1) * NB)
            x_hi = xin.tile([128, NB], B16, tag="xhi")
            x_lo = xin.tile([68, NB], B16, tag="xlo")
            EPS = epool.tile([32, NB], B16, tag="eps")
            nc.sync.dma_start(x_hi[:], xT[0:128, cs])
            nc.sync.dma_start(x_lo[:], xT[128:196, cs])
            nc.sync.dma_start(EPS[:], epsT[:, cs])

            # ---- feature attention ----
            th = thp.tile([98, NB], B16, tag="th98")
            run(mm_stage("fa1", [(W["Wfa1a"][:], x_hi, slice(0, 128)),
                                 (W["Wfa1b"][:], x_lo, slice(0, 68))],
                         98, AF.Tanh, th, W["bfa1"][:]))
            th2_hi = big1.tile([128, NB], B16, tag="big1b")
            run(mm_stage("fa2h", [(W["Wfa2h"][:], th, slice(0, 98))],
                         128, AF.Tanh, th2_hi, W["bfa2h"][:], scale=0.5))
            run(mm_stage("fa2l", [(W["Wfa2l"][:], th, slice(0, 98))],
                         68, AF.Tanh, th2_lo, W["bfa2l"][:], scale=0.5,
                         out_rows=slice(0, 68)))
            # 2*mean(a) replicated over 128 partitions
            rcp = big1.tile([128, NB], F32, tag="big1")
            for g, pairs in GR:
                ps = psA.tile([128, 1024], F32, name=f"ps_mb{g}", tag="psA")
                for psl, nsl in pairs:
                    nc.tensor.matmul(ps[:, psl], r(W["onesA"][:]),
                                     r(th2_hi[:, nsl]), start=True, stop=False)
                    nc.tensor.matmul(ps[:, psl], r(W["onesB"][:]),
                                     r(th2_lo[0:69, nsl]), start=False, stop=True)
                nc.vector.reciprocal(rcp[:, (2 * g) * 512:(2 * g + 2) * 512], ps[:])
            # attn = (th2+1) * (1/(2m));  x_attn = x * attn
            attn_hi = aout.tile([128, NB], F32, tag="athi")
            attn_lo = aout.tile([68, NB], F32, tag="atlo")
            nc.vector.scalar_tensor_tensor(attn_hi[:], th2_hi[:], 1.0, rcp[:],
                                           op0=ALU.add, op1=ALU.mult)
            nc.vector.scalar_tensor_tensor(attn_lo[:], th2_lo[0:68, :], 1.0,
                                           rcp[0:68, :], op0=ALU.add, op1=ALU.mult)
            nc.sync.dma_start(attnT[0:128, cs], attn_hi[:])
            nc.sync.dma_start(attnT[128:196, cs], attn_lo[:])
            xa_hi = big1.tile([128, NB], B16, tag="big1b")
            xa_lo = lo1.tile([68, NB], B16, tag="lo1b")
            nc.vector.tensor_tensor(xa_hi[:], x_hi[:], attn_hi[:], op=ALU.mult)
            nc.vector.tensor_tensor(xa_lo[:], x_lo[:], attn_lo[:], op=ALU.mult)

            # ---- encoder ----
            h1 = thp.tile([96, NB], B16, tag="th98")
            run(mm_stage("e1", [(W["We1a"][:], xa_hi, slice(0, 128)),
                                (W["We1b"][:], xa_lo, slice(0, 68))],
                         96, AF.Prelu, h1, W["be1"][:], alpha=ALPHA))
            h23 = misc.tile([96, NB], B16, tag="h23")
            run(mm_stage("e2", [(W["We2"][:], h1, slice(0, 96))],
                         64, AF.Prelu, h23, W["be2"][:], alpha=ALPHA,
                         out_rows=slice(0, 64)))
            run(mm_stage("e3", [(W["We3"][:], h23, slice(0, 64))],
                         32, AF.Prelu, h23, W["be3"][:], alpha=ALPHA,
                         out_rows=slice(64, 96)))
            # mean/logvar (+replicas)
            zin = misc.tile([40, NB], B16, tag="zin")
            std4 = misc.tile([32, NB], B16, tag="std4")
            mvout = misc.tile([16, NB], F32, tag="mvout")
            for g, pairs in GR:
                ps = psA.tile([96, 1024], F32, name=f"ps_mv{g}", tag="psA")
                for psl, nsl in pairs:
                    nc.tensor.matmul(ps[:, psl], r(W["Wmv"][64:96, :]),
                                     r(h23[64:96, nsl]), start=True, stop=True)
                gs = slice((2 * g) * 512, (2 * g + 2) * 512)
                nc.scalar.activation(mvout[:, gs], ps[0:16, :], AF.Identity,
                                     bias=W["bmv"][:])
                nc.scalar.copy(zin[32:40, gs], ps[32:40, :])
                nc.scalar.activation(std4[:, gs], ps[64:96, :], AF.Exp,
                                     bias=W["bv4h"][:], scale=0.5)
            nc.sync.dma_start(meanT[:, cs], mvout[0:8, :])
            nc.sync.dma_start(logvarT[:, cs], mvout[8:16, :])
            nc.vector.tensor_tensor(zin[0:32, :], EPS[:], std4[:], op=ALU.mult)

            # ---- fused 4-stream decoder ----
            L1 = big1.tile([128, NB], B16, tag="big1b")
            run(mm_stage("d1", [(W["Wd1x"][:], zin, slice(0, 40))],
                         128, AF.Prelu, L1, W["bd1x"][:], alpha=ALPHA))
            L2a = decp.tile([128, NB], B16, tag="l2a")
            L2b = decp.tile([128, NB], B16, tag="l2b")
            run(mm_stage("d2a", [(W["Wd2x"][0:64, :], L1, slice(0, 64))],
                         128, AF.Prelu, L2a, W["bd2x"][:], alpha=ALPHA))
            run(mm_stage("d2b", [(W["Wd2x"][64:128, :], L1, slice(64, 128))],
                         128, AF.Prelu, L2b, W["bd2x"][:], alpha=ALPHA))
            # streams: 0 -> recon, 1 -> h1-loss, 2 -> h2-loss, 3 -> h3-loss
            s0 = decp.tile([96, NB], B16, tag="l3s0")
            s1 = decp.tile([96, NB], B16, tag="l3s1")
            s2 = decp.tile([96, NB], B16, tag="l3s2")
            s3 = decp.tile([96, NB], B16, tag="l3s3")
            run(mm_stage("d3s0", [(W["Wd3x"][0:64, :], L2a, slice(0, 64))],
                         96, AF.Prelu, s0, W["bd3"][:], alpha=ALPHA))
            run(mm_stage("d3s1", [(W["Wd3x"][64:128, :], L2a, slice(64, 128))],
                         96, AF.Prelu, s1, W["bd3"][:], alpha=ALPHA))
            run(mm_stage("d3s2", [(W["Wd3x"][0:64, :], L2b, slice(0, 64))],
                         96, AF.Prelu, s2, W["bd3"][:], alpha=ALPHA))
            run(mm_stage("d3s3", [(W["Wd3x"][64:128, :], L2b, slice(64, 128))],
                         96, AF.Prelu, s3, W["bd3"][:], alpha=ALPHA))

            # recon (stream 0): sigma(x) = 0.5*tanh(0.5x+0.5b)+0.5
            threc_hi = big1.tile([128, NB], F32, tag="big1")
            threc_lo = lo1.tile([68, NB], F32, tag="lo1")
            run(mm_stage("rech", [(W["Wd4h"][:], s0, slice(0, 96))],
                         128, AF.Tanh, threc_hi, W["brech"][:], scale=0.5))
            run(mm_stage("recl", [(W["Wd4l"][:], s0, slice(0, 96))],
                         68, AF.Tanh, threc_lo, W["brecl"][:], scale=0.5))
            rec_hi = rout.tile([128, NB], F32, tag="rechi")
            rec_lo = rout.tile([68, NB], F32, tag="reclo")
            nc.vector.tensor_scalar(rec_hi[:], threc_hi[:], 0.5, 0.5,
                                    op0=ALU.mult, op1=ALU.add)
            nc.vector.tensor_scalar(rec_lo[:], threc_lo[:], 0.5, 0.5,
                                    op0=ALU.mult, op1=ALU.add)
            nc.sync.dma_start(reconT[0:128, cs], rec_hi[:])
            nc.sync.dma_start(reconT[128:196, cs], rec_lo[:])

            # per-layer losses: sq = (h - (0.5*thl+0.5))^2
            thl1 = sqp.tile([96, NB], F32, tag="thl1")
            run(mm_stage("lr1", [(W["Wd4_96"][:], s1, slice(0, 96))],
                         96, AF.Tanh, thl1, W["bl1h"][:], scale=0.5))
            thl23 = sqp.tile([96, NB], F32, tag="thl23")
            run(mm_stage("lr2", [(W["Wd4_64"][:], s2, slice(0, 96))],
                         64, AF.Tanh, thl23, W["bl23h"][0:64, :], scale=0.5,
                         out_rows=slice(0, 64)))
            run(mm_stage("lr3", [(W["Wd4_32"][:], s3, slice(0, 96))],
                         32, AF.Tanh, thl23, W["bl23h"][64:96, :], scale=0.5,
                         out_rows=slice(64, 96)))
            d2_1 = sqp.tile([96, NB], F32, tag="d21")
            d2_23 = sqp.tile([96, NB], F32, tag="d223")
            nc.vector.scalar_tensor_tensor(d2_1[:], h1[:], 2.0, thl1[:],
                                           op0=ALU.mult, op1=ALU.subtract)
            nc.vector.scalar_tensor_tensor(d2_23[:], h23[:], 2.0, thl23[:],
                                           op0=ALU.mult, op1=ALU.subtract)
            sq1 = sqp.tile([96, NB], B16, tag="sq1")
            sq23 = sqp.tile([96, NB], B16, tag="sq23")
            nc.scalar.activation(sq1[:], d2_1[:], AF.Square,
                                 bias=W["bneg05"][:], scale=0.5)
            nc.scalar.activation(sq23[:], d2_23[:], AF.Square,
                                 bias=W["bneg05"][:], scale=0.5)

            # layer losses + layer attention (tiny partition counts)
            lls = lap.tile([3, NB], B16, tag="lls")
            for c, csl in CH:
                pl = psS.tile([3, 512], F32, name=f"ps_lls{c}", tag="psS")
                nc.tensor.matmul(pl[:, :], r(W["O96"][:]), r(sq1[:, csl]),
                                 start=True, stop=False)
                nc.tensor.matmul(pl[:, :], r(W["O64"][:]), r(sq23[0:64, csl]),
                                 start=False, stop=False)
                nc.tensor.matmul(pl[:, :], r(W["O32"][64:96, :]),
                                 r(sq23[64:96, csl]), start=False, stop=True)
                nc.scalar.copy(lls[:, csl], pl[:])
            tla = lap.tile([3, NB], F32, tag="tla")
            for c, csl in CH:
                pa = psS.tile([3, 512], F32, name=f"ps_la{c}", tag="psS")
                nc.tensor.matmul(pa[:, :], r(W["Wla"][:]), r(lls[:, csl]),
                                 start=True, stop=True)
                nc.scalar.activation(tla[:, csl], pa[:], AF.Tanh,
                                     bias=W["blah"][:], scale=0.5)
            # e = (1+t)/(1-t);  wl = sum(ll*e)/sum(e)
            vla = lap.tile([3, NB], F32, tag="vla")
            nc.vector.tensor_scalar(vla[:], tla[:], -1.0, 1.0,
                                    op0=ALU.mult, op1=ALU.add)
            nc.vector.reciprocal(vla[:], vla[:])
            nc.vector.scalar_tensor_tensor(est[0:3, :], tla[:], 1.0, vla[:],
                                           op0=ALU.add, op1=ALU.mult)
            nc.vector.tensor_tensor(est[32:35, :], lls[:], est[0:3, :],
                                    op=ALU.mult)
            for c, csl in CH:
                pse = psS.tile([33, 512], F32, name=f"ps_se{c}", tag="psS")
                nc.tensor.matmul(pse[:, :], r(W["onesE"][:]), r(est[:, csl]),
                                 start=True, stop=True)
                nc.vector.reciprocal(junk[:], pse[0:1, :])
                nc.vector.scalar_tensor_tensor(
                    junk[:], pse[32:33, :], 1.0, junk[:],
                    op0=ALU.mult, op1=ALU.mult,
                    accum_out=wl_acc[:, t * NC_CH + c: t * NC_CH + c + 1])

        # final: per-core sum of weighted layer loss
        wl_sum = const_pool.tile([1, 1], F32, tag="wlsum")
        nc.vector.tensor_reduce(wl_sum[:], wl_acc[:], axis=mybir.AxisListType.X,
                                op=mybir.AluOpType.add)
        nc.sync.dma_start(wlout[:], wl_sum[:])

    nc.compile()
    return nc


def _get_built():
    global _BUILT
    if _BUILT is None:
        _BUILT = _build()
    return _BUILT


def kernel(**inputs):
    from concourse.bass_utils import run_bass_kernel_spmd

    nc = _get_built()
    w = _prep_host(inputs)

    x = np.asarray(inputs["x"], np.float32)
    # eps exactly as the reference draws it
    import jax
    cpu = jax.devices("cpu")[0]
    with jax.default_device(cpu):
        ek = jax.random.split(jax.random.key(42), 4)
        eps = [np.asarray(jax.random.normal(ek[k], (B, 8), np.float32))
               for k in range(4)]

    xTc = np.ascontiguousarray(
        x.reshape(NCORES, BC, D).transpose(0, 2, 1)).astype(BF)   # [8,196,BC]
    epsTc = np.ascontiguousarray(
        np.stack([e.reshape(NCORES, BC, 8) for e in eps], 1)
        .transpose(0, 1, 3, 2).reshape(NCORES, 32, BC)).astype(BF)

    in_maps = []
    for c in range(NCORES):
        m = {"xT": xTc[c], "epsT": epsTc[c]}
        m.update(w)
        in_maps.append(m)

    res = run_bass_kernel_spmd(nc, in_maps, core_ids=list(range(NCORES)))

    recon = np.empty((B, D), np.float32)
    attn = np.empty((B, D), np.float32)
    mean = np.empty((B, 8), np.float32)
    logvar = np.empty((B, 8), np.float32)
    wl = 0.0
    for c in range(NCORES):
        o = res.results[c]
        sl = slice(c * BC, (c + 1) * BC)
        recon[sl] = o["reconT"].T
        attn[sl] = o["attnT"].T
        mean[sl] = o["meanT"].T
        logvar[sl] = o["logvarT"].T
        wl += float(o["wlout"][0, 0])
    wll = np.float32(wl / B)
    return recon, mean, logvar, attn, wll
